# revision 18
# baseline (speedup 1.0000x reference)
"""Multi-level dense 3D conv (AbstractConv3D) as a Trainium2 Bass kernel.

Strategy
--------
Each level l is a dense r^3 grid with a 3x3x3 conv (16->16 ch, zero pad).
In linear (x-major) token space the conv is a 27-tap shifted-window stencil:
    out[t] = sum_k W_k^T x[t + dd_k] + bias,  dd_k = dx*P^2 + dy*P + dz
once the grid is zero-padded so that border taps read zeros.

Host (numpy): pad each level to [8s+2, P, P] with P=r+1 (shared y/z pad
row/col + zero x-halo slabs), cast to bf16, transpose to channel-major
[16, T] (plus a row of ones for fused bias), and shard the x-dimension 8
ways so every core runs an *identical* program on different data.

Device (Bass/Tile): for each level, 4 token-range chunks are DMA'd into
partition groups 32q (rows 32q..32q+16). The 3x3x3 conv is 27 accumulating
matmuls per 512-token window, each reading a shifted window of the chunk
(zero-copy taps: just a free-dim offset). tile_position packing (32x32
mode) runs 16 (chunk q, window j) tiles concurrently in the PE array;
window (q, j) accumulates in PSUM bank q, partitions 32j. A row of ones in
the input (partition 32q+16) and a bias row in the first tap's weights
fold the bias add into the matmul. PSUM is evicted by two big split
copies (ScalarE + VectorE) into bf16 staging, then DMA'd to a channel-major
output that the host un-pads/transposes back.
"""

import math
from contextlib import ExitStack

import numpy as np
import ml_dtypes

import concourse.bass as bass
import concourse.bacc as bacc
import concourse.mybir as mybir
import concourse.tile as tile
from concourse.bass_utils import run_bass_kernel_spmd

BF16 = ml_dtypes.bfloat16

RES = [16, 18, 20, 23, 26, 29, 32, 36, 40, 45, 50, 56, 63, 70, 76, 80]
L = 16
CIN = 16
COUT = 16
NCORES = 8
NWIN = 512  # matmul free dim / PSUM bank (f32)

# Per-level geometry
S_L = [math.ceil(r / 8) for r in RES]  # x-slabs per core
P_L = [r + 1 for r in RES]  # padded y/z extent
T_L = [(s + 2) * p * p for s, p in zip(S_L, P_L)]  # piece tokens (w/ x-halo)
H_L = [p * p + p + 1 for p in P_L]  # max |tap shift|
GUARD = 6656  # > max(H_L) = 6643
GAP = 256  # zero gap between levels: valid outputs read up to P+1 before level start
T_IN = GUARD + sum(T_L) + (L - 1) * GAP + GUARD
T_OUT = sum(T_L)
LVL_IN_BASE = [GUARD + sum(T_L[:i]) + i * GAP for i in range(L)]
LVL_OUT_BASE = [sum(T_L[:i]) for i in range(L)]

# 27 taps, row-major (dx, dy, dz) matching weight.reshape(27, CIN, COUT)
TAPS = [(dx, dy, dz) for dx in (-1, 0, 1) for dy in (-1, 0, 1) for dz in (-1, 0, 1)]

_CACHE = {}


def _build_program(levels=None, reps=1):
    """One SPMD program, identical for all cores. reps>1 wraps the body in an
    on-device loop (benchmark amplification only)."""
    if levels is None:
        levels = range(L)
    nc = bacc.Bacc("TRN2", target_bir_lowering=False, debug=False, num_devices=NCORES)
    x_ext = nc.declare_dram_parameter("x", [17, T_IN], mybir.dt.bfloat16, isOutput=False)
    w_ext = nc.declare_dram_parameter("w", [128, L * 27 * 16], mybir.dt.bfloat16, isOutput=False)
    out_ext = nc.declare_dram_parameter("o", [16, T_OUT], mybir.dt.bfloat16, isOutput=True)

    with tile.TileContext(nc) as tc, ExitStack() as ctx:
        w_pool = ctx.enter_context(tc.tile_pool(name="w", bufs=1))
        x_pool = ctx.enter_context(tc.tile_pool(name="x", bufs=2))
        ps_pool = ctx.enter_context(tc.tile_pool(name="ps", bufs=2, space="PSUM"))
        st_pool = ctx.enter_context(tc.tile_pool(name="st", bufs=3))

        w_sb = w_pool.tile([128, L * 27 * 16], mybir.dt.bfloat16)
        nc.sync.dma_start(w_sb[:], w_ext[:])

        for _rep in range(reps):
            body_levels = list(levels)
            _emit_body(nc, tc, x_pool, ps_pool, st_pool, w_sb, x_ext, out_ext, body_levels)
    nc.finalize()
    return nc


def _emit_body(nc, tc, x_pool, ps_pool, st_pool, w_sb, x_ext, out_ext, levels):
        for lvl in levels:
            P = P_L[lvl]
            T = T_L[lvl]
            H = H_L[lvl]
            # Compute only the real slabs [P^2, T-P^2): the x-halo slabs are
            # input-only (their "outputs" would be discarded by the host).
            TC = T - 2 * P * P
            Q = math.ceil(TC / 4)  # computed tokens per chunk (last may be smaller)
            qlens = [min(Q, TC - q * Q) for q in range(4)]
            aq = [P * P + q * Q for q in range(4)]  # chunk output start (level coords)
            F = Q + 2 * H  # loaded extent per chunk
            nwin = math.ceil(Q / NWIN)

            xt = x_pool.tile([128, F], mybir.dt.bfloat16, tag="xchunk")
            SEG = 32768  # DMA descriptor rows must stay under 64KB
            for q in range(4):
                a = LVL_IN_BASE[lvl] + aq[q] - H
                fq = qlens[q] + 2 * H
                for s0 in range(0, fq, SEG):
                    sl = min(SEG, fq - s0)
                    nc.sync.dma_start(
                        xt[32 * q : 32 * q + 17, s0 : s0 + sl],
                        x_ext[:, a + s0 : a + s0 + sl],
                    )

            dd = {k: dx * P * P + dy * P + dz for k, (dx, dy, dz) in enumerate(TAPS)}

            for t in range(math.ceil(nwin / 4)):
                ps = ps_pool.tile([128, 4 * NWIN], mybir.dt.float32)
                live = []  # (q, j, w0, nw)
                for q in range(4):
                    for j in range(4):
                        w = t * 4 + j
                        w0 = H + w * NWIN
                        nw = min(NWIN, qlens[q] + H - w0)
                        if nw > 0:
                            live.append((q, j, w0, nw))
                for k in range(27):
                    kk = 17 if k == 0 else 16
                    for q, j, w0, nw in live:
                        lhsT = w_sb[32 * q : 32 * q + kk, (lvl * 27 + k) * 16 : (lvl * 27 + k) * 16 + 16]
                        rhs = xt[32 * q : 32 * q + kk, w0 + dd[k] : w0 + dd[k] + nw]
                        nc.tensor.matmul(
                            ps[32 * j : 32 * j + 16, q * NWIN : q * NWIN + nw],
                            lhsT,
                            rhs,
                            start=(k == 0),
                            stop=(k == 26),
                            tile_position=(32 * q, 32 * j),
                        )
                st = st_pool.tile([128, 4 * NWIN], mybir.dt.bfloat16, tag="stage")
                half = 2 * NWIN
                nc.scalar.copy(st[:, 0:half], ps[:, 0:half])
                nc.vector.tensor_copy(st[:, half : 4 * NWIN], ps[:, half : 4 * NWIN])
                for q, j, w0, nw in live:
                    ob = LVL_OUT_BASE[lvl] + aq[q] + (w0 - H)
                    nc.sync.dma_start(
                        out_ext[:, ob : ob + nw],
                        st[32 * j : 32 * j + 16, q * NWIN : q * NWIN + nw],
                    )


def _pack_inputs(input, weight, bias):
    """Host-side pad/cast/transpose/shard. Returns per-core in_maps."""
    x = np.asarray(input)[0]  # [N, 16] f32
    wt = np.asarray(weight).reshape(L, 27, CIN, COUT)
    bs = np.asarray(bias)

    # Weights: [128, L*27*16] bf16; rows 32q+ci = W[l,k,ci,co], row 32q+16 =
    # bias (folded into tap 0 against the ones input row).
    wb = np.zeros((128, L * 27 * 16), dtype=BF16)
    wrow = wt.transpose(2, 0, 1, 3).reshape(CIN, L * 27 * COUT).astype(BF16)
    brow = np.zeros((L, 27, COUT), dtype=np.float32)
    brow[:, 0, :] = bs
    brow = brow.reshape(L * 27 * COUT).astype(BF16)
    for q in range(4):
        wb[32 * q : 32 * q + 16, :] = wrow
        wb[32 * q + 16, :] = brow

    xs = [np.zeros((17, T_IN), dtype=BF16) for _ in range(NCORES)]
    for xi in xs:
        xi[16, :] = 1.0

    off = 0
    for lvl, r in enumerate(RES):
        P, s = P_L[lvl], S_L[lvl]
        g = x[off : off + r**3].reshape(r, r, r, CIN)
        off += r**3
        gp = np.zeros((CIN, 8 * s + 2, P, P), dtype=BF16)
        gp[:, 1 : r + 1, 0:r, 0:r] = g.transpose(3, 0, 1, 2)
        for i in range(NCORES):
            piece = gp[:, i * s : i * s + s + 2].reshape(CIN, T_L[lvl])
            xs[i][0:16, LVL_IN_BASE[lvl] : LVL_IN_BASE[lvl] + T_L[lvl]] = piece

    return [{"x": xs[i], "w": wb} for i in range(NCORES)]


def _unpack_outputs(results):
    """Assemble [1, N, 16] f32 from per-core padded channel-major outputs."""
    n_total = sum(r**3 for r in RES)
    out = np.empty((1, n_total, CIN), dtype=np.float32)
    off = 0
    for lvl, r in enumerate(RES):
        P, s = P_L[lvl], S_L[lvl]
        for i in range(NCORES):
            n_i = min(s, r - i * s)
            if n_i <= 0:
                continue
            o = np.asarray(results[i]["o"], dtype=np.float32)
            piece = o[:, LVL_OUT_BASE[lvl] : LVL_OUT_BASE[lvl] + T_L[lvl]]
            piece = piece.reshape(CIN, s + 2, P, P)[:, 1 : 1 + n_i, 0:r, 0:r]
            dst = off + i * s * r * r
            out[0, dst : dst + n_i * r * r] = piece.transpose(1, 2, 3, 0).reshape(-1, CIN)
        off += r**3
    return out


def run(input, offsets, resolutions, weight, bias, trace=False, levels=None, **trace_kw):
    key = ("nc", tuple(levels) if levels is not None else None)
    if key not in _CACHE:
        _CACHE[key] = _build_program(levels)
    nc = _CACHE[key]
    in_maps = _pack_inputs(input, weight, bias)
    res = run_bass_kernel_spmd(nc, in_maps, list(range(NCORES)), trace=trace, **trace_kw)
    return _unpack_outputs(res.results), res


def kernel(input, offsets, resolutions, weight, bias):
    out, _ = run(input, offsets, resolutions, weight, bias)
    return out
